# revision 1
# baseline (speedup 1.0000x reference)
"""Trainium2 Bass kernel for CustomEncoderWithAction (gnn_message_passing).

Strategy (8 NeuronCores, full inputs in / full output out):
  * Only pooled[robot_idx] (B=192 rows) is consumed downstream, so the
    [N,N] pairwise pooling is computed ONLY for the 192 robot agents,
    sharded 24 per core.
  * Pooling layer-1 decomposes: u1[i,j,:] = a[i,:] + b[j,:] with
      a_i = pos_i @ Wc,  b_j = -pos_j @ Wc + h_j @ W_p1b + (b_sp@W_p1a + b_p1),
      Wc = W_sp @ W_p1[:EMB]   (weight folding done host-side).
    relu(a_i + b_j) is one tensor_scalar/activation op per pair of robots.
  * Layer-2 (64->16) runs on the TensorEngine with a block-diagonal
    [128, 32] weight (2 robots per pass), 8 robots' z stacked in PSUM [128, N].
  * Neighbor mask + max-pool fused into one DVE tensor_tensor_reduce per
    robot group: accum = max_j (z + mask) with mask = -2^30 on non-neighbors;
    pooled = relu(max + b_p2) since relu/bias commute with max and
    no-neighbor rows come out 0 automatically.
  * LSTM encoder (T=8, all N agents) is replicated on every core, N-major:
    lhsT = [hT; traj_rel_t; 1] tiles (x-embedding folded: W_he@W_ih),
    sigmoid/tanh on ACT, state update split across DVE and GPSIMD,
    h transposed back to feature-major via PE transposes; elementwise in
    halves to pipeline.  Step 0 skips the h/c terms entirely (h0=c0=0).
  * All small constants ship in one [128, CB] bf16 blob (single DMA).
"""

import numpy as np
import ml_dtypes
from contextlib import ExitStack

import concourse.bass as bass
import concourse.bacc as bacc
import concourse.tile as tile
from concourse import mybir
from concourse.bass_utils import run_bass_kernel_spmd

F32 = mybir.dt.float32
BF16 = mybir.dt.bfloat16
AL = mybir.AluOpType
AF = mybir.ActivationFunctionType
AX = mybir.AxisListType

T, N, B, A_DIM, H, EMB, MID, F = 8, 1536, 192, 2, 16, 16, 64, 256
NC_ = 8          # cores
BPC = B // NC_   # 24 robots per core
NPAIR = BPC // 2  # 12
NGRP = BPC // 8   # 3
BIG = float(2 ** 30)
CH = 512          # psum free chunk
NCH = N // CH     # 3
HK = 4            # LSTM tiles per lane (3 lanes)

bf16 = ml_dtypes.bfloat16

# blob column layout (bf16 [128, CB]); cols 0:CA = LSTM-critical (first DMA)
_C_WCAT2 = 0        # [19, 64]  gate weights (i,f,o,g cols)
_C_WCAT0 = 64       # [3, 64]   step-0 gate weights (x rows + bias)
_C_IDENT = 128      # [128, 128]
CA = 128
_C_WB2 = 256        # [19, 128] bT2 stationary, 2 col-copies
_C_BD = 384         # [128, 32]
_C_WCP = 416        # [2, 64]
_C_PICE = 480       # [2, NPAIR]
_C_PICO = 492       # [2, NPAIR]
_C_WEMB = 504       # [4, 16]
_C_WFCA = 520       # [48, 256]
_C_SEL = 776        # [128, 128]  (8 x [128,16] identity slices)
_C_SPT = 904        # [4, BPC]
_C_MSEL = 928       # [8, 128]
CB = 1056


def _din(nc, name, shape, dt):
    return nc.dram_tensor(name, list(shape), dt, kind="ExternalInput").ap()


_IN_SPECS = [
    ("trajposT", [3, T + 1, N], BF16),
    ("t0x", [3, N], BF16),
    ("identI", [128, 128], BF16),
    ("nm8", [8, NGRP, N], BF16),
    ("blobA", [128, CA], BF16),
    ("blobC", [128, CB - CA], BF16),
    ("blobF", [128, 2], F32),
    ("rpo", [16, BPC], BF16),
]


def _build():
    nc = bacc.Bacc("TRN2", target_bir_lowering=False, debug=False)
    a = {nm: _din(nc, nm, sh, dt) for nm, sh, dt in _IN_SPECS}
    a["out"] = nc.dram_tensor("out", [BPC, F], F32, kind="ExternalOutput").ap()
    with tile.TileContext(nc) as tc, ExitStack() as ctx:
        _emit(ctx, tc, nc, a)
    nc.compile()
    return nc


def _emit(ctx, tc, nc, a):
    sb = ctx.enter_context(tc.tile_pool(name="sb", bufs=1))

    # prefetch the sigmoid/tanh ACT table set immediately (scalar queue kept
    # free of DMAs so the table loads run under the input-DMA window)
    warm = sb.tile([1, 2], F32, name="warm")
    nc.vector.memset(warm, 0.0)
    nc.scalar.activation(out=warm, in_=warm, func=AF.Sigmoid)
    nc.scalar.activation(out=warm, in_=warm, func=AF.Tanh)

    # ---------- input DMAs: sync + gpsimd queues only ----------
    xh = sb.tile([35, T + 1, N], BF16, name="xh")
    blob = sb.tile([128, CB], BF16, name="blob")
    identI = sb.tile([128, 128], BF16, name="identI")
    nc.sync.dma_start(out=xh[32:35, 0, :], in_=a["t0x"])
    nc.gpsimd.dma_start(out=blob[:, 0:CA], in_=a["blobA"])
    nc.sync.dma_start(out=identI, in_=a["identI"])
    nc.gpsimd.dma_start(out=blob[:, CA:CB], in_=a["blobC"])
    # rows 16:19 for all T steps plus slot T: posT/ones for the bT2 lhsT
    nc.sync.dma_start(out=xh[16:19, :, :], in_=a["trajposT"])
    ench = xh[0:19, T, :]   # rows 0:16 h_T, 16:18 posT, 18 ones

    nm_sb = sb.tile([8, NGRP, N], BF16, name="nm_sb")
    nc.gpsimd.dma_start(out=nm_sb, in_=a["nm8"])

    blobF = sb.tile([128, 2], F32, name="blobF")
    nc.sync.dma_start(out=blobF, in_=a["blobF"])

    fuseT = sb.tile([48, BPC], BF16, name="fuseT")
    nc.sync.dma_start(out=fuseT[16:32, :], in_=a["rpo"])


    W_cat2 = blob[0:19, _C_WCAT2:_C_WCAT2 + 64]
    W_cat0 = blob[32:35, _C_WCAT0:_C_WCAT0 + 64]
    msel = blob[0:8, _C_MSEL:_C_MSEL + 128]
    Wb2 = blob[0:19, _C_WB2:_C_WB2 + 128]
    ident = identI[:, :]
    BD_sb = blob[:, _C_BD:_C_BD + 32]
    WcP_sb = blob[0:2, _C_WCP:_C_WCP + 64]
    pIcE_sb = blob[0:2, _C_PICE:_C_PICE + NPAIR]
    pIcO_sb = blob[0:2, _C_PICO:_C_PICO + NPAIR]
    W_emb_sb = blob[0:4, _C_WEMB:_C_WEMB + H]
    W_fca_sb = blob[0:48, _C_WFCA:_C_WFCA + F]
    spT = blob[0:4, _C_SPT:_C_SPT + BPC]
    b_embT = blobF[0:16, 0:1]
    b_p2T = blobF[32:48, 1:2]

    # ---------- LSTM over T steps (replicated, all N agents) ----------
    c_sb = sb.tile([128, 12, H], BF16, name="c_sb")
    sg = sb.tile([128, 12, 48], BF16, name="sg")
    tg = sb.tile([128, 12, H], BF16, name="tg")
    th = sb.tile([128, 12, H], BF16, name="th")
    hn = sb.tile([128, 12, H], BF16, name="hn")
    t1 = sb.tile([128, 12, H], BF16, name="t1")
    t2 = sb.tile([128, 12, H], BF16, name="t2")

    with tc.tile_pool(name="lstm_g", bufs=2, space="PSUM") as gpool, \
         tc.tile_pool(name="lstm_tp", bufs=4, space="PSUM") as tpool:
        for t in range(T):
            g_ps = gpool.tile([128, 12, 4 * H], F32, name="g_ps")
            for half in range(12 // HK):
                hs = slice(HK * half, HK * (half + 1))
                for k in range(HK * half, HK * (half + 1)):
                    if t == 0:
                        nc.tensor.matmul(
                            g_ps[:, k, :], xh[32:35, 0, 128 * k:128 * (k + 1)],
                            W_cat0, start=True, stop=True)
                    else:
                        nc.tensor.matmul(
                            g_ps[:, k, :], xh[0:19, t, 128 * k:128 * (k + 1)],
                            W_cat2, start=True, stop=True)
                # gate cols: [i(0:16), f(16:32), o(32:48), g(48:64)]
                nc.scalar.activation(
                    out=sg[:, hs, :], in_=g_ps[:, hs, 0:48], func=AF.Sigmoid)
                nc.scalar.activation(
                    out=tg[:, hs, :], in_=g_ps[:, hs, 48:64], func=AF.Tanh)
                if t == 0:
                    # c = i * g  (c0 = 0)
                    nc.vector.tensor_tensor(
                        out=c_sb[:, hs, :], in0=sg[:, hs, 0:16],
                        in1=tg[:, hs, :], op=AL.mult)
                else:
                    nc.vector.tensor_tensor(
                        out=t1[:, hs, :], in0=sg[:, hs, 0:16], in1=tg[:, hs, :],
                        op=AL.mult)
                    nc.vector.tensor_tensor(
                        out=t2[:, hs, :], in0=sg[:, hs, 16:32],
                        in1=c_sb[:, hs, :], op=AL.mult)
                    nc.vector.tensor_tensor(
                        out=c_sb[:, hs, :], in0=t1[:, hs, :], in1=t2[:, hs, :],
                        op=AL.add)
                nc.scalar.activation(
                    out=th[:, hs, :], in_=c_sb[:, hs, :], func=AF.Tanh)
                nc.vector.tensor_tensor(
                    out=hn[:, hs, :], in0=sg[:, hs, 32:48], in1=th[:, hs, :],
                    op=AL.mult)
                tp = tpool.tile([H, HK * 128], BF16, name="tp")
                for q in range(HK):
                    nc.tensor.transpose(
                        tp[:, 128 * q:128 * (q + 1)],
                        hn[:, HK * half + q, :], ident)
                half_off = HK * 128 * half
                dest = xh[0:16, t + 1, :]
                nc.vector.tensor_copy(
                    out=dest[:, half_off:half_off + HK * 128], in_=tp)

    # ---------- bT2 [128, N]: b_j stacked twice on partitions ----------
    bT2 = sb.tile([128, N], BF16, name="bT2")
    aT2 = sb.tile([128, NPAIR], F32, name="aT2")
    with tc.tile_pool(name="b_ps", bufs=2, space="PSUM") as bpool:
        for ch in range(NCH):
            s = slice(CH * ch, CH * (ch + 1))
            b_ps = bpool.tile([128, CH], F32, name="b_ps")
            nc.tensor.matmul(b_ps, Wb2, xh[0:19, T, s], start=True, stop=True)
            nc.scalar.copy(out=bT2[:, s], in_=b_ps)

        a_ps = bpool.tile([128, NPAIR], F32, name="a_ps")
        nc.tensor.matmul(a_ps[0:64, :], WcP_sb, pIcE_sb, start=True, stop=True)
        nc.tensor.matmul(a_ps[64:128, :], WcP_sb, pIcO_sb, start=True, stop=True)
        nc.vector.tensor_copy(out=aT2, in_=a_ps)

    # ---------- pairwise pooling ----------
    pool_parts = sb.tile([128, NGRP], BF16, name="pool_parts")
    with tc.tile_pool(name="ru_pool", bufs=4) as rupool, \
         tc.tile_pool(name="z_ps", bufs=2, space="PSUM") as zpool:
        for g in range(NGRP):
            z = zpool.tile([128, N], F32, name="z", tag="z")
            for ch in range(NCH):
                s = slice(CH * ch, CH * (ch + 1))
                nc.tensor.matmul(
                    z[:, s], msel, nm_sb[:, g, s], start=True, stop=False,
                    skip_group_check=True)
            for ai in range(4):
                p = 4 * g + ai
                ru = rupool.tile([128, N], BF16, name="ru", tag="ru")
                if ai < 2 or (g == 0 and ai == 2):
                    nc.vector.tensor_scalar(
                        out=ru, in0=bT2, scalar1=aT2[:, p:p + 1], scalar2=0.0,
                        op0=AL.add, op1=AL.max)
                else:
                    nc.scalar.activation(
                        out=ru, in_=bT2, func=AF.Relu,
                        bias=aT2[:, p:p + 1])
                for ch in range(NCH):
                    s = slice(CH * ch, CH * (ch + 1))
                    nc.tensor.matmul(
                        z[32 * ai:32 * (ai + 1), s], BD_sb, ru[:, s],
                        start=False, stop=(ai == 3),
                        tile_position=(0, 32 * ai), skip_group_check=True)
            nc.vector.tensor_reduce(
                out=pool_parts[:, g:g + 1], in_=z, axis=AX.X, op=AL.max)

    # ---------- fusion MLP for this core's 24 robots ----------
    out_sb = sb.tile([BPC, F], F32, name="out_sb")
    with tc.tile_pool(name="f_ps", bufs=1, space="PSUM") as fpool:
        se_ps = fpool.tile([H, BPC], F32, name="se_ps")
        nc.tensor.matmul(se_ps, W_emb_sb, spT, start=True, stop=True)
        nc.scalar.activation(
            out=fuseT[0:16, :], in_=se_ps, func=AF.Relu, bias=b_embT)
        pg_ps = fpool.tile([48, BPC], F32, name="pg_ps")
        pg_v = pg_ps.rearrange("p (c l) -> p c l", l=8)
        for l in range(8):
            nc.tensor.matmul(
                pg_v[32:48, :, l], blob[:, _C_SEL + 16 * l:_C_SEL + 16 * (l + 1)],
                pool_parts, start=True, stop=True)
        nc.scalar.activation(
            out=fuseT[32:48, :], in_=pg_ps[32:48, :], func=AF.Relu,
            bias=b_p2T)
        o_ps = fpool.tile([BPC, F], F32, name="o_ps")
        nc.tensor.matmul(o_ps, fuseT, W_fca_sb, start=True, stop=True)
        nc.scalar.activation(out=out_sb, in_=o_ps, func=AF.Relu)
    nc.sync.dma_start(out=a["out"], in_=out_sb)


# ------------------------------------------------------------------
# host side
# ------------------------------------------------------------------
_NC_CACHE = None


def _gates_reorder(w):
    # torch gate order i,f,g,o (16 each) -> i,f,o,g
    i, f, g, o = np.split(np.asarray(w, np.float32), 4, axis=-1)
    return np.concatenate([i, f, o, g], axis=-1)


def _bf(x):
    return np.ascontiguousarray(np.asarray(x, np.float32).astype(bf16))


def _f32(x):
    return np.ascontiguousarray(np.asarray(x, np.float32))


def kernel(obs_traj_pos, traj_rel, neigh_index, robot_idx, r_goal, r_pose,
           action, W_he, b_he, W_ih, W_hh, b_ih, b_hh, W_sp, b_sp, W_p1, b_p1,
           W_p2, b_p2, W_emb, b_emb, W_fc, b_fc):
    global _NC_CACHE
    obs_traj_pos = np.asarray(obs_traj_pos, np.float32)
    traj_rel = np.asarray(traj_rel, np.float32)
    neigh_index = np.asarray(neigh_index)
    robot_idx = np.asarray(robot_idx)
    pos = obs_traj_pos[-1]                        # [N, 2]
    f = _f32

    # fold x-embedding into the recurrent matmul:
    #   gates = traj_rel@(W_he W_ih) + h@W_hh + (b_ih + b_he@W_ih + b_hh)
    W_heih = f(W_he) @ f(W_ih)
    bias = f(b_ih) + f(b_he) @ f(W_ih) + f(b_hh)
    W_cat2 = np.zeros((19, 64), np.float32)
    W_cat2[0:16] = _gates_reorder(W_hh)
    W_cat2[16:18] = _gates_reorder(W_heih)
    W_cat2[18] = _gates_reorder(bias)
    W_cat0 = W_cat2[16:19]                        # step-0: x rows + bias only

    Wc = f(W_sp) @ f(W_p1)[:EMB]                  # [2, 64]
    cvec = f(b_sp) @ f(W_p1)[:EMB] + f(b_p1)      # [64]
    # bT2 stationary: ench rows [h(16); posT(2); ones(1)]
    Wb2h = np.zeros((19, 64), np.float32)
    Wb2h[0:16] = f(W_p1)[EMB:]
    Wb2h[16:18] = -Wc
    Wb2h[18] = cvec
    Wb2 = np.concatenate([Wb2h, Wb2h], axis=1)    # [19, 128]

    blob0 = np.zeros((128, CB), np.float32)
    blob0[0:19, _C_WCAT2:_C_WCAT2 + 64] = W_cat2
    blob0[32:35, _C_WCAT0:_C_WCAT0 + 64] = W_cat0
    blob0[0:19, _C_WB2:_C_WB2 + 128] = Wb2
    bd = np.zeros((128, 32), np.float32)
    bd[0:64, 0:16] = W_p2
    bd[64:128, 16:32] = W_p2
    blob0[:, _C_BD:_C_BD + 32] = bd
    blob0[0:2, _C_WCP:_C_WCP + 64] = Wc
    blob0[0:4, _C_WEMB:_C_WEMB + H] = W_emb
    wf = np.zeros((48, F), np.float32)
    wf[0:16] = W_fc[0:16]        # spatial_emb rows
    wf[16:21] = W_fc[32:37]      # r_pose rows
    wf[21] = b_fc                # bias row (matched by ones in rpo row 5->21)
    wf[32:48] = W_fc[16:32]      # pooled rows
    blob0[0:48, _C_WFCA:_C_WFCA + F] = wf
    blob0[:, _C_SEL:_C_SEL + 128] = np.eye(128)

    trajT = np.concatenate(
        [np.transpose(traj_rel, (2, 0, 1)),
         np.ones((1, T, N), np.float32)], axis=0)      # [3, T, N]
    posT3 = np.concatenate(
        [pos.T, np.ones((1, N), np.float32)], axis=0)  # [3, N]
    trajposT = _bf(np.concatenate([trajT, posT3[:, None, :]], axis=1))
    blobF0 = np.zeros((128, 2), np.float32)
    blobF0[0:16, 0] = f(b_emb)
    blobF0[32:48, 1] = f(b_p2)

    ms = np.zeros((8, 128), np.float32)
    for l in range(8):
        ms[l, 16 * l:16 * (l + 1)] = -BIG
    blob0[0:8, _C_MSEL:_C_MSEL + 128] = ms
    in_maps = []
    for c in range(NC_):
        I = robot_idx[BPC * c:BPC * (c + 1)]
        nm = np.zeros((8, NGRP, N), np.float32)
        for g in range(NGRP):
            for l in range(8):
                nm[l, g] = 1.0 - (neigh_index[I[8 * g + l]] > 0)
        blobc = blob0.copy()
        blobc[0:2, _C_PICE:_C_PICE + NPAIR] = pos[I[0::2]].T
        blobc[0:2, _C_PICO:_C_PICO + NPAIR] = pos[I[1::2]].T
        spt = np.zeros((4, BPC), np.float32)
        spt[0:2] = (f(r_goal)[BPC * c:BPC * (c + 1)] - pos[I]).T
        spt[2:4] = f(action)[BPC * c:BPC * (c + 1)].T
        blobc[0:4, _C_SPT:_C_SPT + BPC] = spt
        rpo = np.zeros((16, BPC), np.float32)
        rpo[0:5] = f(r_pose)[BPC * c:BPC * (c + 1)].T
        rpo[5] = 1.0
        in_maps.append(dict(
            trajposT=trajposT,
            t0x=np.ascontiguousarray(trajposT[:, 0, :]),
            identI=_bf(np.eye(128)),
            nm8=_bf(nm),
            blobA=_bf(blobc[:, :CA]),
            blobC=_bf(blobc[:, CA:]),
            blobF=blobF0,
            rpo=_bf(rpo),
        ))

    if _NC_CACHE is None:
        _NC_CACHE = _build()
    res = run_bass_kernel_spmd(_NC_CACHE, in_maps, core_ids=list(range(NC_)))
    out = np.concatenate([r["out"] for r in res.results], axis=0)
    return out.astype(np.float32)


if __name__ == "__main__":
    import reference
    inp = {k: np.asarray(v) for k, v in reference.setup_inputs().items()}
    got = kernel(**inp)
    exp = np.asarray(reference.reference(**inp))
    err = np.abs(got - exp)
    print("max abs err", err.max(), "scale", np.abs(exp).max())
    print("rel-of-max", err.max() / np.abs(exp).max())



# revision 19
# speedup vs baseline: 1.1256x; 1.1256x over previous
"""Trainium2 Bass kernel for CustomEncoderWithAction (gnn_message_passing).

Strategy (8 NeuronCores, full inputs in / full output out):
  * Only pooled[robot_idx] (B=192 rows) is consumed downstream, so the
    [N,N] pairwise pooling is computed ONLY for the 192 robot agents,
    sharded 24 per core.
  * LSTM encoder (T=8, all N agents) replicated on every core, software-
    pipelined as 2 groups x 6 agent-tiles:
      - per group+step ONE gates matmul: stationary lhsT = transposed
        [h|x|1] block (one PE transpose per group), moving rhs = [109,384]
        block-column weight matrix (tile j's W_hh/W_xh rows live at row
        offset 18j; zeros elsewhere kill cross-tile terms).
      - batched activations: one sigmoid [128,6,48] + two tanh per group
        (the ~352cyc ACT pipeline fill made small per-tile activations the
        baseline bottleneck).
      - h-update writes straight into the next step's transpose input.
  * Pooling layer-1 decomposes: u1[i,j,:] = a[i,:] + b[j,:] (weight folding
    host-side); relu(a_i + b_j) one DVE/ACT op per robot pair; layer-2 on
    PE with block-diagonal [128,32] weight; neighbor mask folded into the
    PSUM accumulator via a -2^30 selection matmul; masked max-pool =
    tensor_reduce; pooled = relu(max + b_p2).
  * Fusion MLP on-device per core (24 robots).
"""

import numpy as np
import ml_dtypes
from contextlib import ExitStack

import concourse.bass as bass
import concourse.bacc as bacc
import concourse.tile as tile
from concourse import mybir
from concourse.bass_utils import run_bass_kernel_spmd

F32 = mybir.dt.float32
BF16 = mybir.dt.bfloat16
AL = mybir.AluOpType
AF = mybir.ActivationFunctionType
AX = mybir.AxisListType

T, N, B, A_DIM, H, EMB, MID, F = 8, 1536, 192, 2, 16, 16, 64, 256
NC_ = 8          # cores
BPC = B // NC_   # 24 robots per core
NPAIR = BPC // 2  # 12
NGRP = BPC // 8   # 3 robot groups of 8
BIG = float(2 ** 30)
CH = 512          # psum free chunk
NCH = N // CH     # 3
NT = N // 128     # 12 agent tiles
GT = 6            # tiles per LSTM group
TC = H + 2        # trin cols per tile (h16 + x2)
TW = GT * TC + 1  # 109: 6 tiles + ones col

bf16 = ml_dtypes.bfloat16

# blob column layout (bf16 [128, CB]); matmul operands all at partition 0
_C_BD = 0        # rows 0:128  [128, 32]
_C_WB2 = 32      # rows 0:99   [99, 128] bT2 stationary (h rows at 16j), 2 col-copies
_C_MSEL = 160    # rows 0:8    [8, 128]
_C_WCP = 288     # rows 0:2    [2, 64]
_C_PICE = 352    # rows 0:2    [2, 12]
_C_PICO = 364    # rows 0:2    [2, 12]
_C_WEMB = 376    # rows 0:4    [4, 16]
_C_SPT = 392     # rows 0:4    [4, 24]
_C_WFCA = 416    # rows 0:48   [48, 256]
CB = 672
NH = N // 2      # 768 agent columns per LSTM group


def _din(nc, name, shape, dt):
    return nc.dram_tensor(name, list(shape), dt, kind="ExternalInput").ap()


_IN_SPECS = [
    ("xa", [128, NT, T, 2], BF16),
    ("identI", [128, 128], BF16),
    ("wall", [128, GT * 64], BF16),
    ("posT3", [3, N], BF16),
    ("nm8", [8, NGRP, N], BF16),
    ("blobB", [128, CB], BF16),
    ("blobF", [128, 2], F32),
    ("rpo", [16, BPC], BF16),
]


def _build():
    nc = bacc.Bacc("TRN2", target_bir_lowering=False, debug=False)
    a = {nm: _din(nc, nm, sh, dt) for nm, sh, dt in _IN_SPECS}
    a["out"] = nc.dram_tensor("out", [BPC, F], F32, kind="ExternalOutput").ap()
    with tile.TileContext(nc) as tc, ExitStack() as ctx:
        _emit(ctx, tc, nc, a)
    nc.compile()
    return nc


def _emit(ctx, tc, nc, a):
    sb = ctx.enter_context(tc.tile_pool(name="sb", bufs=1))

    # prefetch the sigmoid/tanh ACT table set immediately
    warm = sb.tile([1, 2], F32, name="warm")
    nc.vector.memset(warm, 0.0)
    nc.scalar.activation(out=warm, in_=warm, func=AF.Sigmoid)
    nc.scalar.activation(out=warm, in_=warm, func=AF.Tanh)

    # ---------- input DMAs ----------
    blob = sb.tile([128, CB], BF16, name="blob")
    identI = sb.tile([128, 128], BF16, name="identI")
    xa = sb.tile([128, NT, T, 2], BF16, name="xa")
    wall = sb.tile([128, GT, 64], BF16, name="wall")
    ench = sb.tile([19, N], BF16, name="ench")  # rows 0:16 hT, 16:18 posT, 18 ones
    nm_sb = sb.tile([8, NGRP, N], BF16, name="nm_sb")
    blobF = sb.tile([128, 2], F32, name="blobF")
    fuseT = sb.tile([48, BPC], BF16, name="fuseT")

    nc.sync.dma_start(out=xa, in_=a["xa"])
    nc.sync.dma_start(out=identI, in_=a["identI"])
    nc.gpsimd.dma_start(out=wall, in_=a["wall"])
    nc.gpsimd.dma_start(out=blob, in_=a["blobB"])
    nc.sync.dma_start(out=ench[16:19, :], in_=a["posT3"])
    nc.gpsimd.dma_start(out=nm_sb, in_=a["nm8"])
    nc.sync.dma_start(out=blobF, in_=a["blobF"])
    nc.sync.dma_start(out=fuseT[16:32, :], in_=a["rpo"])

    BD_sb = blob[:, _C_BD:_C_BD + 32]
    Wb2 = blob[0:19, _C_WB2:_C_WB2 + 128]
    msel = blob[0:8, _C_MSEL:_C_MSEL + 128]
    WcP_sb = blob[0:2, _C_WCP:_C_WCP + 64]
    pIcE_sb = blob[0:2, _C_PICE:_C_PICE + NPAIR]
    pIcO_sb = blob[0:2, _C_PICO:_C_PICO + NPAIR]
    W_emb_sb = blob[0:4, _C_WEMB:_C_WEMB + H]
    spT = blob[0:4, _C_SPT:_C_SPT + BPC]
    W_fca_sb = blob[0:48, _C_WFCA:_C_WFCA + F]
    b_embT = blobF[0:16, 0:1]
    b_p2T = blobF[32:48, 1:2]

    # ---------- LSTM state + staging (all bf16 SBUF) ----------
    trin = sb.tile([128, 2, 2, TW], BF16, name="trin")   # [agents, parity, grp, cols]
    c_sb = sb.tile([128, 2, GT, H], BF16, name="c_sb")
    nc.vector.memset(trin, 0.0)
    nc.vector.memset(trin[:, :, :, TW - 1:TW], 1.0)
    nc.vector.memset(c_sb, 0.0)

    aT2 = sb.tile([128, NPAIR], F32, name="aT2")
    with tc.tile_pool(name="init_ps", bufs=1, space="PSUM") as ipool:
        a_ps = ipool.tile([128, NPAIR], F32, name="a_ps")
        nc.tensor.matmul(a_ps[0:64, :], WcP_sb, pIcE_sb, start=True, stop=True)
        nc.tensor.matmul(a_ps[64:128, :], WcP_sb, pIcO_sb, start=True, stop=True)
        nc.vector.tensor_copy(out=aT2, in_=a_ps)
        se_ps = ipool.tile([H, BPC], F32, name="se_ps")
        nc.tensor.matmul(se_ps, W_emb_sb, spT, start=True, stop=True)
        nc.scalar.activation(
            out=fuseT[0:16, :], in_=se_ps, func=AF.Relu, bias=b_embT)

    # ---------- LSTM over T steps, 2 pipelined groups of 6 tiles ----------
    sgp = ctx.enter_context(tc.tile_pool(name="sgp", bufs=2))
    with tc.tile_pool(name="lstm_g", bufs=3, space="PSUM") as gpool, \
         tc.tile_pool(name="lstm_tp", bufs=3, space="PSUM") as tpool, \
         tc.tile_pool(name="lstm_tt", bufs=3) as ttpool:
        for t in range(T):
            par, nxt = t % 2, (t + 1) % 2
            tps, tts, gps, sgs, tgs, ths, t1s, t2s = ([None, None] for _ in range(8))
            # x columns for this step
            for g in range(2):
                hx = trin[:, par, g, 0:GT * TC].rearrange(
                    "p (j c) -> p j c", c=TC)
                nc.vector.tensor_copy(
                    out=hx[:, :, H:H + 2], in_=xa[:, GT * g:GT * g + GT, t, :])
            # PE: transposes then gates matmuls (both groups back-to-back)
            for g in range(2):
                tps[g] = tpool.tile([TW, 128], BF16, name="tp", tag="tp")
                nc.tensor.transpose(tps[g], trin[:, par, g, :], identI)
            for g in range(2):
                tts[g] = ttpool.tile([TW, 128], BF16, name="tt", tag="tt")
                nc.vector.tensor_copy(out=tts[g], in_=tps[g])
            for g in range(2):
                gps[g] = gpool.tile([128, GT, 64], F32, name="g_ps", tag="g_ps")
                nc.tensor.matmul(
                    gps[g], tts[g], wall[0:TW, :, :], start=True, stop=True)
            # ACT: batched sigmoid + tanh(g) per group
            for g in range(2):
                sgs[g] = sgp.tile([128, GT, 48], BF16, name="sg", tag="sg")
                tgs[g] = sgp.tile([128, GT, H], BF16, name="tg", tag="tg")
                nc.scalar.activation(
                    out=sgs[g], in_=gps[g][:, :, 0:48], func=AF.Sigmoid)
                nc.scalar.activation(
                    out=tgs[g], in_=gps[g][:, :, 48:64], func=AF.Tanh)
            # DVE: c update
            for g in range(2):
                t1s[g] = sgp.tile([128, GT, H], BF16, name="t1", tag="t1")
                t2s[g] = sgp.tile([128, GT, H], BF16, name="t2", tag="t2")
                nc.vector.tensor_tensor(
                    out=t1s[g], in0=sgs[g][:, :, 0:16], in1=tgs[g], op=AL.mult)
                nc.vector.tensor_tensor(
                    out=t2s[g], in0=sgs[g][:, :, 16:32], in1=c_sb[:, g, :, :],
                    op=AL.mult)
                nc.vector.tensor_tensor(
                    out=c_sb[:, g, :, :], in0=t1s[g], in1=t2s[g], op=AL.add)
            # ACT: tanh(c); DVE: h -> next parity trin
            for g in range(2):
                ths[g] = sgp.tile([128, GT, H], BF16, name="th", tag="th")
                nc.scalar.activation(
                    out=ths[g], in_=c_sb[:, g, :, :], func=AF.Tanh)
            for g in range(2):
                hx_n = trin[:, nxt, g, 0:GT * TC].rearrange(
                    "p (j c) -> p j c", c=TC)
                nc.vector.tensor_tensor(
                    out=hx_n[:, :, 0:H], in0=sgs[g][:, :, 32:48], in1=ths[g],
                    op=AL.mult)

        # final hT (parity 0 after t=7) -> feature-major ench rows 0:16
        for g in range(2):
            hx_f = trin[:, T % 2, g, 0:GT * TC].rearrange("p (j c) -> p j c", c=TC)
            for jl in range(GT):
                j = GT * g + jl
                tp2 = tpool.tile([H, 128], BF16, name="tp2", tag="tp")
                nc.tensor.transpose(tp2, hx_f[:, jl, 0:H], identI)
                nc.vector.tensor_copy(
                    out=ench[0:16, 128 * j:128 * (j + 1)], in_=tp2)

    # ---------- bT2 [128, N]: b_j stacked twice on partitions ----------
    bT2 = sb.tile([128, N], BF16, name="bT2")
    with tc.tile_pool(name="b_ps", bufs=2, space="PSUM") as bpool:
        for ch in range(NCH):
            s = slice(CH * ch, CH * (ch + 1))
            b_ps = bpool.tile([128, CH], F32, name="b_ps")
            nc.tensor.matmul(b_ps, Wb2, ench[:, s], start=True, stop=True)
            nc.scalar.copy(out=bT2[:, s], in_=b_ps)

    # ---------- pairwise pooling ----------
    pool_parts = sb.tile([128, NGRP], BF16, name="pool_parts")
    with tc.tile_pool(name="ru_pool", bufs=4) as rupool, \
         tc.tile_pool(name="z_ps", bufs=2, space="PSUM") as zpool:
        for g in range(NGRP):
            z = zpool.tile([128, N], F32, name="z", tag="z")
            for ch in range(NCH):
                s = slice(CH * ch, CH * (ch + 1))
                nc.tensor.matmul(
                    z[:, s], msel, nm_sb[:, g, s], start=True, stop=False,
                    skip_group_check=True)
            for ai in range(4):
                p = 4 * g + ai
                ru = rupool.tile([128, N], BF16, name="ru", tag="ru")
                if ai < 2 or (g == 0 and ai == 2):
                    nc.vector.tensor_scalar(
                        out=ru, in0=bT2, scalar1=aT2[:, p:p + 1], scalar2=0.0,
                        op0=AL.add, op1=AL.max)
                else:
                    nc.scalar.activation(
                        out=ru, in_=bT2, func=AF.Relu,
                        bias=aT2[:, p:p + 1])
                for ch in range(NCH):
                    s = slice(CH * ch, CH * (ch + 1))
                    nc.tensor.matmul(
                        z[32 * ai:32 * (ai + 1), s], BD_sb, ru[:, s],
                        start=False, stop=(ai == 3),
                        tile_position=(0, 32 * ai), skip_group_check=True)
            nc.vector.tensor_reduce(
                out=pool_parts[:, g:g + 1], in_=z, axis=AX.X, op=AL.max)

    # ---------- fusion MLP for this core's 24 robots ----------
    out_sb = sb.tile([BPC, F], F32, name="out_sb")
    with tc.tile_pool(name="f_ps", bufs=1, space="PSUM") as fpool:
        pg_ps = fpool.tile([48, BPC], F32, name="pg_ps")
        pg_v = pg_ps.rearrange("p (c l) -> p c l", l=8)
        for l in range(8):
            nc.tensor.matmul(
                pg_v[32:48, :, l], identI[:, 16 * l:16 * (l + 1)],
                pool_parts, start=True, stop=True)
        nc.scalar.activation(
            out=fuseT[32:48, :], in_=pg_ps[32:48, :], func=AF.Relu,
            bias=b_p2T)
        o_ps = fpool.tile([BPC, F], F32, name="o_ps")
        nc.tensor.matmul(o_ps, fuseT, W_fca_sb, start=True, stop=True)
        nc.scalar.activation(out=out_sb, in_=o_ps, func=AF.Relu)
    nc.sync.dma_start(out=a["out"], in_=out_sb)


# ------------------------------------------------------------------
# host side
# ------------------------------------------------------------------
_NC_CACHE = None


def _gates_reorder(w):
    # torch gate order i,f,g,o (16 each) -> i,f,o,g
    i, f, g, o = np.split(np.asarray(w, np.float32), 4, axis=-1)
    return np.concatenate([i, f, o, g], axis=-1)


def _bf(x):
    return np.ascontiguousarray(np.asarray(x, np.float32).astype(bf16))


def _f32(x):
    return np.ascontiguousarray(np.asarray(x, np.float32))


def kernel(obs_traj_pos, traj_rel, neigh_index, robot_idx, r_goal, r_pose,
           action, W_he, b_he, W_ih, W_hh, b_ih, b_hh, W_sp, b_sp, W_p1, b_p1,
           W_p2, b_p2, W_emb, b_emb, W_fc, b_fc):
    global _NC_CACHE
    obs_traj_pos = np.asarray(obs_traj_pos, np.float32)
    traj_rel = np.asarray(traj_rel, np.float32)
    neigh_index = np.asarray(neigh_index)
    robot_idx = np.asarray(robot_idx)
    pos = obs_traj_pos[-1]                        # [N, 2]
    f = _f32

    # fold x-embedding into the recurrent matmul:
    #   gates = traj_rel@(W_he W_ih) + h@W_hh + (b_ih + b_he@W_ih + b_hh)
    W_heih = f(W_he) @ f(W_ih)
    bias = f(b_ih) + f(b_he) @ f(W_ih) + f(b_hh)
    W_cat = np.zeros((19, 64), np.float32)
    W_cat[0:16] = _gates_reorder(W_hh)
    W_cat[16:18] = _gates_reorder(W_heih)
    W_cat[18] = _gates_reorder(bias)

    Wc = f(W_sp) @ f(W_p1)[:EMB]                  # [2, 64]
    cvec = f(b_sp) @ f(W_p1)[:EMB] + f(b_p1)      # [64]
    # bT2 stationary: ench rows [h(16); posT(2); ones(1)]
    Wb2h = np.zeros((19, 64), np.float32)
    Wb2h[0:16] = f(W_p1)[EMB:]
    Wb2h[16:18] = -Wc
    Wb2h[18] = cvec
    Wb2 = np.concatenate([Wb2h, Wb2h], axis=1)    # [19, 128]

    # block-column gate weights: tile j's rows at 18j (zeros elsewhere
    # kill cross-tile terms of the shared transposed lhsT)
    wall0 = np.zeros((128, GT, 64), np.float32)
    for j in range(GT):
        wall0[TC * j:TC * j + 18, j, :] = W_cat[0:18]
        wall0[TW - 1, j, :] = W_cat[18]

    blob0 = np.zeros((128, CB), np.float32)
    bd = np.zeros((128, 32), np.float32)
    bd[0:64, 0:16] = W_p2
    bd[64:128, 16:32] = W_p2
    blob0[:, _C_BD:_C_BD + 32] = bd
    blob0[0:19, _C_WB2:_C_WB2 + 128] = Wb2
    ms = np.zeros((8, 128), np.float32)
    for l in range(8):
        ms[l, 16 * l:16 * (l + 1)] = -BIG
    blob0[0:8, _C_MSEL:_C_MSEL + 128] = ms
    blob0[0:2, _C_WCP:_C_WCP + 64] = Wc
    blob0[0:4, _C_WEMB:_C_WEMB + H] = W_emb
    wf = np.zeros((48, F), np.float32)
    wf[0:16] = W_fc[0:16]        # spatial_emb rows
    wf[16:21] = W_fc[32:37]      # r_pose rows
    wf[21] = b_fc                # bias row (matched by ones in rpo row 5->21)
    wf[32:48] = W_fc[16:32]      # pooled rows
    blob0[0:48, _C_WFCA:_C_WFCA + F] = wf

    # agent-major traj_rel: xa[a, j, t, :] = traj_rel[t, 128j+a, :]
    xa = np.transpose(traj_rel.reshape(T, NT, 128, 2), (2, 1, 0, 3))
    posT3 = np.concatenate(
        [pos.T, np.ones((1, N), np.float32)], axis=0)  # [3, N]
    blobF0 = np.zeros((128, 2), np.float32)
    blobF0[0:16, 0] = f(b_emb)
    blobF0[32:48, 1] = f(b_p2)

    in_maps = []
    for c in range(NC_):
        I = robot_idx[BPC * c:BPC * (c + 1)]
        nm = np.zeros((8, NGRP, N), np.float32)
        for g in range(NGRP):
            for l in range(8):
                nm[l, g] = 1.0 - (neigh_index[I[8 * g + l]] > 0)
        blobc = blob0.copy()
        blobc[0:2, _C_PICE:_C_PICE + NPAIR] = pos[I[0::2]].T
        blobc[0:2, _C_PICO:_C_PICO + NPAIR] = pos[I[1::2]].T
        spt = np.zeros((4, BPC), np.float32)
        spt[0:2] = (f(r_goal)[BPC * c:BPC * (c + 1)] - pos[I]).T
        spt[2:4] = f(action)[BPC * c:BPC * (c + 1)].T
        blobc[0:4, _C_SPT:_C_SPT + BPC] = spt
        rpo = np.zeros((16, BPC), np.float32)
        rpo[0:5] = f(r_pose)[BPC * c:BPC * (c + 1)].T
        rpo[5] = 1.0
        in_maps.append(dict(
            xa=_bf(xa),
            identI=_bf(np.eye(128)),
            wall=_bf(wall0.reshape(128, GT * 64)),
            posT3=_bf(posT3),
            nm8=_bf(nm),
            blobB=_bf(blobc),
            blobF=blobF0,
            rpo=_bf(rpo),
        ))

    if _NC_CACHE is None:
        _NC_CACHE = _build()
    res = run_bass_kernel_spmd(_NC_CACHE, in_maps, core_ids=list(range(NC_)))
    out = np.concatenate([r["out"] for r in res.results], axis=0)
    return out.astype(np.float32)


if __name__ == "__main__":
    import reference
    inp = {k: np.asarray(v) for k, v in reference.setup_inputs().items()}
    got = kernel(**inp)
    exp = np.asarray(reference.reference(**inp))
    err = np.abs(got - exp)
    print("max abs err", err.max(), "scale", np.abs(exp).max())
    print("rel-of-max", err.max() / np.abs(exp).max())
